# revision 3
# baseline (speedup 1.0000x reference)
"""LoRA linear layer on 8 Trainium2 NeuronCores.

Computes out = x @ (lora_B @ lora_A * 2).T + bias for
x [4, 2048, 4096], lora_A [16, 4096], lora_B [4096, 16], bias [4096].

Strategy: pure data parallel — shard x over batch*seq (8192 rows -> 1024
rows/core), replicate the tiny LoRA weights. Per core, exploit the rank-16
structure: y = x @ A^T (contract 4096), z = y @ B^T * 2 + bias (contract 16).

v3 (memory-regime tuning):
  * All device-side matmul traffic is bf16 (1 PE cycle/row vs 4 for fp32;
    rel-err budget is 2e-2, bf16 lands ~5e-3). Output stored bf16 and
    upcast on the host. Per-core HBM traffic: 8 MiB in + 8 MiB out.
  * The host pre-transposes each x shard to x^T AND pre-tiles it
    partition-major ([row-block][partition][k-chunk*rows]) so every block
    load is 128 descriptors x 8 KiB contiguous — full HBM line rate.
    No PE transposes, no transpose PSUM round-trip.
  * 8 row-blocks of 128 rows keep PE idle gaps well under the ~3.4 us HAM
    re-throttle window; warm-up matmuls during the first load bring the
    PE clock to 2.4 GHz before real work starts.

Per-core pipeline over 8 row-blocks of 128 rows:
  1. One DMA pulls the block's x^T [128, 32x128] into SBUF (8 KiB runs).
  2. GEMM1: 32 accumulating matmuls, lhsT = (2A)^T chunk [128,16] bf16,
     rhs = x^T chunk [128,128] bf16 -> y^T [16,128] fp32 in PSUM.
  3. Bias trick: yt = [y^T; ones] [17,128] bf16; BB = [B^T; bias] [17,4096]
     bf16. GEMM2: z chunk [128,512] — bias is added by the matmul itself.
  4. Copy z PSUM -> SBUF bf16 (alternating ScalarE/VectorE), DMA out
     [128, 4096] (1 MiB contiguous).
"""

import sys

import numpy as np

if "/opt/trn_rl_repo" not in sys.path:
    sys.path.insert(0, "/opt/trn_rl_repo")

import ml_dtypes

import concourse.bass as bass
import concourse.mybir as mybir
from concourse import bacc
from concourse.bass_utils import run_bass_kernel_spmd
from concourse.tile import TileContext

N_CORES = 8
B, S, IN_F, OUT_F, R = 4, 2048, 4096, 4096, 16
ROWS = B * S // N_CORES  # 1024 rows per core
SCALING = 2.0  # alpha / r = 32 / 16
FP32 = mybir.dt.float32
BF16 = mybir.dt.bfloat16
BF = ml_dtypes.bfloat16
P = 128
NK = IN_F // P  # 32 contraction chunks for GEMM1
RB = 128  # rows per pipelined block
NB = ROWS // RB  # 8 blocks per core
ZC = 512  # GEMM2 moving chunk (one PSUM bank of fp32)
NJ = OUT_F // ZC  # 8 output chunks per row tile
NWARM = 10  # HAM warm-up matmuls (~3.6 us cold — one SHORT window)

_nc_cache = None


def build_nc() -> bass.Bass:
    nc = bacc.Bacc()
    # x^T, pre-tiled partition-major by the host: row-block nb's data sits
    # in rows [nb*P, (nb+1)*P), with chunk k at cols [k*RB, (k+1)*RB).
    xt_d = nc.declare_dram_parameter("xt", [NB * P, NK * RB], BF16, isOutput=False)
    # (2A)^T, same partition-major chunk tiling: [128, 32*16]
    at_d = nc.declare_dram_parameter("at", [P, NK * R], BF16, isOutput=False)
    bb_d = nc.declare_dram_parameter("bb", [R + 1, OUT_F], BF16, isOutput=False)
    out_d = nc.declare_dram_parameter("out", [ROWS, OUT_F], BF16, isOutput=True)

    with TileContext(nc) as tc:
        with (
            tc.tile_pool(name="const", bufs=1) as const,
            tc.tile_pool(name="xin", bufs=4) as xin,
            tc.tile_pool(name="ytp", bufs=2) as ytp,
            tc.tile_pool(name="zrp", bufs=3) as zrp,
            tc.tile_pool(name="wpsum", bufs=1, space="PSUM") as wpsum,
            tc.tile_pool(name="ypsum", bufs=2, space="PSUM") as ypsum,
            tc.tile_pool(name="zpsum", bufs=4, space="PSUM") as zpsum,
        ):
            # HAM warm-up: keep the PE busy through one full activity window
            # while the first x block loads, so real matmuls run at 2.4 GHz.
            wsrc = const.tile([P, ZC], BF16)
            nc.vector.memset(wsrc[:, :], 0.0)
            w_ps = wpsum.tile([P, ZC], FP32)
            for _ in range(NWARM):
                nc.tensor.matmul(
                    w_ps, lhsT=wsrc[:, :P], rhs=wsrc[:, :], start=True, stop=True
                )

            at_sb = const.tile([P, NK * R], BF16)
            nc.sync.dma_start(out=at_sb[:, :], in_=at_d[:, :])
            bb = const.tile([R + 1, OUT_F], BF16)
            nc.sync.dma_start(out=bb[:, :], in_=bb_d[:, :])

            for nb in range(NB):
                xt_sb = xin.tile([P, NK * RB], BF16, tag="x")
                nc.sync.dma_start(
                    out=xt_sb[:, :], in_=xt_d[nb * P : (nb + 1) * P, :]
                )

                y_ps = ypsum.tile([R, RB], FP32, tag="y")
                for k in range(NK):
                    nc.tensor.matmul(
                        y_ps,
                        lhsT=at_sb[:, k * R : (k + 1) * R],
                        rhs=xt_sb[:, k * RB : (k + 1) * RB],
                        start=(k == 0),
                        stop=(k == NK - 1),
                    )

                # Ones-fill the whole tile (engines can't start at partition
                # 16), then overwrite rows 0:16 with y — row 16 keeps the 1.0.
                yt_sb = ytp.tile([R + 1, RB], BF16, tag="yt")
                nc.vector.memset(yt_sb[:, :], 1.0)
                nc.scalar.copy(out=yt_sb[0:R, :], in_=y_ps)

                zrow = zrp.tile([P, OUT_F], BF16, tag="z")
                for j in range(NJ):
                    z_ps = zpsum.tile([P, ZC], FP32, tag="zz")
                    nc.tensor.matmul(
                        z_ps,
                        lhsT=yt_sb[:, :],
                        rhs=bb[:, j * ZC : (j + 1) * ZC],
                        start=True,
                        stop=True,
                    )
                    dst = zrow[:, j * ZC : (j + 1) * ZC]
                    if j % 2 == 0:
                        nc.vector.tensor_copy(out=dst, in_=z_ps)
                    else:
                        nc.scalar.copy(out=dst, in_=z_ps)
                nc.sync.dma_start(
                    out=out_d[nb * P : (nb + 1) * P, :], in_=zrow[:, :]
                )

    nc.finalize()  # Bacc.finalize runs compile(): wait legalization + reg alloc
    return nc


def make_in_maps(x, lora_A, lora_B, bias):
    x2 = np.asarray(x, dtype=np.float32).reshape(B * S, IN_F)
    # (2A)^T [4096, 16] -> partition-major chunk tiling [128, 32*16]
    at = (np.asarray(lora_A, dtype=np.float32).T * SCALING).astype(BF)
    at = np.ascontiguousarray(
        at.reshape(NK, P, R).transpose(1, 0, 2).reshape(P, NK * R)
    )
    bbh = np.ascontiguousarray(
        np.concatenate(
            [
                np.asarray(lora_B, dtype=np.float32).T,
                np.asarray(bias, dtype=np.float32)[None, :],
            ],
            axis=0,
        ).astype(BF)
    )
    xb = x2.astype(BF)
    maps = []
    for s in np.split(xb, N_CORES, axis=0):
        # s [1024 rows, 4096] -> x^T [4096, 1024] -> [NB*P, NK*RB]
        # partition-major: element (k*P+p, nb*RB+r) -> (nb*P+p, k*RB+r)
        xt = np.ascontiguousarray(
            s.T.reshape(NK, P, NB, RB)
            .transpose(2, 1, 0, 3)
            .reshape(NB * P, NK * RB)
        )
        maps.append({"xt": xt, "at": at, "bb": bbh})
    return maps


def run(inputs: dict, trace: bool = False, **kw):
    global _nc_cache
    if _nc_cache is None:
        _nc_cache = build_nc()
    in_maps = make_in_maps(**inputs)
    res = run_bass_kernel_spmd(
        _nc_cache, in_maps, list(range(N_CORES)), trace=trace, **kw
    )
    out = (
        np.concatenate([res.results[i]["out"] for i in range(N_CORES)], axis=0)
        .astype(np.float32)
        .reshape(B, S, OUT_F)
    )
    return out, res


def kernel(**inputs) -> np.ndarray:
    out, _ = run(inputs)
    return out


# revision 4
# speedup vs baseline: 1.1489x; 1.1489x over previous
"""LoRA linear layer on 8 Trainium2 NeuronCores.

Computes out = x @ (lora_B @ lora_A * 2).T + bias for
x [4, 2048, 4096], lora_A [16, 4096], lora_B [4096, 16], bias [4096].

Strategy: pure data parallel — shard x over batch*seq (8192 rows -> 1024
rows/core), replicate the tiny LoRA weights. Per core, exploit the rank-16
structure: y = x @ A^T (contract 4096), z = y @ B^T * 2 + bias (contract 16).

v4 (memory-regime tuning):
  * All device-side matmul traffic is bf16 (1 PE cycle/row vs 4 for fp32;
    rel-err budget is 2e-2, bf16 lands ~5e-3). Output stored bf16 and
    upcast on the host. Per-core HBM traffic: 8 MiB in + 8 MiB out.
  * Host pre-transposes each x shard to x^T and pre-tiles it so each
    1 MiB load piece is a fully contiguous [128, 8x512] DRAM slab
    (8 KiB per partition line -> line-rate descriptors). No PE
    transposes, no transpose PSUM round-trip.
  * SDMA engines round-robin between all in-flight transfers at packet
    granularity, so a deep prefetch queue delays the FIRST transfer's
    completion. Loads are issued on the SP HWDGE ring in exact consume
    order with a 3-piece pool bound, stores + const loads go on the
    separate ACT ring so they never head-of-line-block a load issue.
  * 512-row blocks -> 64 GEMM1 + 64 GEMM2 matmuls, all N=512 (per-MM
    issue/LDWEIGHTS overhead amortized); HAM warm-up matmuls during the
    first load so real matmuls run at 2.4 GHz.

Per-core pipeline: 2 row-blocks of 512 rows, each loaded as 4 x 1 MiB
pieces (8 k-chunks per piece):
  1. GEMM1: 32 accumulating matmuls per block, lhsT = (2A)^T chunk
     [128,16] bf16, rhs = x^T chunk [128,512] -> y^T [16,512] fp32 PSUM.
  2. Bias trick: yt = [y^T; ones] [17,512] bf16; BB = [B^T; bias]
     [17,4096] bf16. GEMM2 per 128-row tile: 8 matmuls [128,512] — bias
     is added by the matmul itself.
  3. Copy z PSUM -> SBUF bf16 (alternating ScalarE/VectorE), DMA out
     [128, 4096] (1 MiB contiguous) on the ACT ring.
"""

import sys

import numpy as np

if "/opt/trn_rl_repo" not in sys.path:
    sys.path.insert(0, "/opt/trn_rl_repo")

import ml_dtypes

import concourse.bass as bass
import concourse.mybir as mybir
from concourse import bacc
from concourse.bass_utils import run_bass_kernel_spmd
from concourse.tile import TileContext

N_CORES = 8
B, S, IN_F, OUT_F, R = 4, 2048, 4096, 4096, 16
ROWS = B * S // N_CORES  # 1024 rows per core
SCALING = 2.0  # alpha / r = 32 / 16
FP32 = mybir.dt.float32
BF16 = mybir.dt.bfloat16
BF = ml_dtypes.bfloat16
P = 128
NK = IN_F // P  # 32 contraction chunks for GEMM1
RB = 512  # rows per block (one PSUM bank of fp32 y^T)
NB = ROWS // RB  # 2 blocks per core
KQ = 8  # k-chunks per load piece
NQ = NK // KQ  # 4 load pieces per block
HT = RB // P  # 4 row-tiles per block
ZC = 512  # GEMM2 moving chunk (one PSUM bank of fp32)
NJ = OUT_F // ZC  # 8 output chunks per row tile
NWARM = 10  # HAM warm-up matmuls (~4 us cold — one SHORT window)

_nc_cache = None


def build_nc() -> bass.Bass:
    nc = bacc.Bacc()
    # x^T, host-tiled: piece (b, q) occupies rows [(b*NQ+q)*P, +P), with
    # chunk kk of the piece at cols [kk*RB, (kk+1)*RB). Fully contiguous.
    xt_d = nc.declare_dram_parameter(
        "xt", [NB * NQ * P, KQ * RB], BF16, isOutput=False
    )
    # (2A)^T, partition-major chunk tiling: [128, 32*16]
    at_d = nc.declare_dram_parameter("at", [P, NK * R], BF16, isOutput=False)
    bb_d = nc.declare_dram_parameter("bb", [R + 1, OUT_F], BF16, isOutput=False)
    out_d = nc.declare_dram_parameter("out", [ROWS, OUT_F], BF16, isOutput=True)

    with TileContext(nc) as tc:
        with (
            tc.tile_pool(name="const", bufs=1) as const,
            tc.tile_pool(name="xin", bufs=3) as xin,
            tc.tile_pool(name="ytp", bufs=2) as ytp,
            tc.tile_pool(name="zrp", bufs=3) as zrp,
            tc.tile_pool(name="wpsum", bufs=1, space="PSUM") as wpsum,
            tc.tile_pool(name="ypsum", bufs=2, space="PSUM") as ypsum,
            tc.tile_pool(name="zpsum", bufs=4, space="PSUM") as zpsum,
        ):
            # HAM warm-up: keep the PE busy through one full activity window
            # while the first x piece loads, so real matmuls run at 2.4 GHz.
            wsrc = const.tile([P, ZC], BF16)
            nc.vector.memset(wsrc[:, :], 0.0)
            w_ps = wpsum.tile([P, ZC], FP32)
            for _ in range(NWARM):
                nc.tensor.matmul(
                    w_ps, lhsT=wsrc[:, :P], rhs=wsrc[:, :], start=True, stop=True
                )

            # Const loads on the ACT ring — keep the SP ring free for x.
            at_sb = const.tile([P, NK * R], BF16)
            nc.scalar.dma_start(out=at_sb[:, :], in_=at_d[:, :])
            bb = const.tile([R + 1, OUT_F], BF16)
            nc.scalar.dma_start(out=bb[:, :], in_=bb_d[:, :])

            for nb in range(NB):
                pieces = []
                for q in range(NQ):
                    xt_p = xin.tile([P, KQ * RB], BF16, tag="x")
                    nc.sync.dma_start(
                        out=xt_p[:, :],
                        in_=xt_d[(nb * NQ + q) * P : (nb * NQ + q + 1) * P, :],
                    )
                    pieces.append(xt_p)

                y_ps = ypsum.tile([R, RB], FP32, tag="y")
                for k in range(NK):
                    q, kk = k // KQ, k % KQ
                    nc.tensor.matmul(
                        y_ps,
                        lhsT=at_sb[:, k * R : (k + 1) * R],
                        rhs=pieces[q][:, kk * RB : (kk + 1) * RB],
                        start=(k == 0),
                        stop=(k == NK - 1),
                    )

                # Ones-fill the whole tile (engines can't start at partition
                # 16), then overwrite rows 0:16 with y — row 16 keeps the 1.0.
                yt_sb = ytp.tile([R + 1, RB], BF16, tag="yt")
                nc.vector.memset(yt_sb[:, :], 1.0)
                nc.scalar.copy(out=yt_sb[0:R, :], in_=y_ps)

                for h in range(HT):
                    zrow = zrp.tile([P, OUT_F], BF16, tag="z")
                    for j in range(NJ):
                        z_ps = zpsum.tile([P, ZC], FP32, tag="zz")
                        nc.tensor.matmul(
                            z_ps,
                            lhsT=yt_sb[:, h * P : (h + 1) * P],
                            rhs=bb[:, j * ZC : (j + 1) * ZC],
                            start=True,
                            stop=True,
                        )
                        dst = zrow[:, j * ZC : (j + 1) * ZC]
                        if j % 2 == 0:
                            nc.vector.tensor_copy(out=dst, in_=z_ps)
                        else:
                            nc.scalar.copy(out=dst, in_=z_ps)
                    nc.scalar.dma_start(
                        out=out_d[(nb * HT + h) * P : (nb * HT + h + 1) * P, :],
                        in_=zrow[:, :],
                    )

    nc.finalize()  # Bacc.finalize runs compile(): wait legalization + reg alloc
    return nc


def make_in_maps(x, lora_A, lora_B, bias):
    x2 = np.asarray(x, dtype=np.float32).reshape(B * S, IN_F)
    # (2A)^T [4096, 16] -> partition-major chunk tiling [128, 32*16]
    at = (np.asarray(lora_A, dtype=np.float32).T * SCALING).astype(BF)
    at = np.ascontiguousarray(
        at.reshape(NK, P, R).transpose(1, 0, 2).reshape(P, NK * R)
    )
    bbh = np.ascontiguousarray(
        np.concatenate(
            [
                np.asarray(lora_B, dtype=np.float32).T,
                np.asarray(bias, dtype=np.float32)[None, :],
            ],
            axis=0,
        ).astype(BF)
    )
    xb = x2.astype(BF)
    maps = []
    for s in np.split(xb, N_CORES, axis=0):
        # s [1024 rows, 4096] -> x^T [4096 = (q kk p), 1024 = (b r)]
        # -> piece-major [(b q p), (kk r)]
        xt = np.ascontiguousarray(
            s.T.reshape(NQ, KQ, P, NB, RB)
            .transpose(3, 0, 2, 1, 4)
            .reshape(NB * NQ * P, KQ * RB)
        )
        maps.append({"xt": xt, "at": at, "bb": bbh})
    return maps


def run(inputs: dict, trace: bool = False, **kw):
    global _nc_cache
    if _nc_cache is None:
        _nc_cache = build_nc()
    in_maps = make_in_maps(**inputs)
    res = run_bass_kernel_spmd(
        _nc_cache, in_maps, list(range(N_CORES)), trace=trace, **kw
    )
    out = (
        np.concatenate([res.results[i]["out"] for i in range(N_CORES)], axis=0)
        .astype(np.float32)
        .reshape(B, S, OUT_F)
    )
    return out, res


def kernel(**inputs) -> np.ndarray:
    out, _ = run(inputs)
    return out


# revision 6
# speedup vs baseline: 1.2405x; 1.0797x over previous
"""LoRA linear layer on 8 Trainium2 NeuronCores.

Computes out = x @ (lora_B @ lora_A * 2).T + bias for
x [4, 2048, 4096], lora_A [16, 4096], lora_B [4096, 16], bias [4096].

Strategy: pure data parallel — shard x over batch*seq (8192 rows -> 1024
rows/core), replicate the tiny LoRA weights. Per core, exploit the rank-16
structure: y = x @ A^T (contract 4096), z = y @ B^T * 2 + bias (contract 16).

v4 (memory-regime tuning):
  * All device-side matmul traffic is bf16 (1 PE cycle/row vs 4 for fp32;
    rel-err budget is 2e-2, bf16 lands ~5e-3). Output stored bf16 and
    upcast on the host. Per-core HBM traffic: 8 MiB in + 8 MiB out.
  * Host pre-transposes each x shard to x^T and pre-tiles it so each
    1 MiB load piece is a fully contiguous [128, 8x512] DRAM slab
    (8 KiB per partition line -> line-rate descriptors). No PE
    transposes, no transpose PSUM round-trip.
  * SDMA engines round-robin between all in-flight transfers at packet
    granularity, so a deep prefetch queue delays the FIRST transfer's
    completion. Loads are issued on the SP HWDGE ring in exact consume
    order with a 3-piece pool bound, stores + const loads go on the
    separate ACT ring so they never head-of-line-block a load issue.
  * 512-row blocks -> 64 GEMM1 + 64 GEMM2 matmuls, all N=512 (per-MM
    issue/LDWEIGHTS overhead amortized); HAM warm-up matmuls during the
    first load so real matmuls run at 2.4 GHz.

Per-core pipeline: 2 row-blocks of 512 rows, each loaded as 4 x 1 MiB
pieces (8 k-chunks per piece):
  1. GEMM1: 32 accumulating matmuls per block, lhsT = (2A)^T chunk
     [128,16] bf16, rhs = x^T chunk [128,512] -> y^T [16,512] fp32 PSUM.
  2. Bias trick: yt = [y^T; ones] [17,512] bf16; BB = [B^T; bias]
     [17,4096] bf16. GEMM2 per 128-row tile: 8 matmuls [128,512] — bias
     is added by the matmul itself.
  3. Copy z PSUM -> SBUF bf16 (alternating ScalarE/VectorE), DMA out
     [128, 4096] (1 MiB contiguous) on the ACT ring.
"""

import sys

import numpy as np

if "/opt/trn_rl_repo" not in sys.path:
    sys.path.insert(0, "/opt/trn_rl_repo")

import ml_dtypes

import concourse.bass as bass
import concourse.mybir as mybir
from concourse import bacc
from concourse.bass_utils import run_bass_kernel_spmd
from concourse.tile import TileContext

N_CORES = 8
B, S, IN_F, OUT_F, R = 4, 2048, 4096, 4096, 16
ROWS = B * S // N_CORES  # 1024 rows per core
SCALING = 2.0  # alpha / r = 32 / 16
FP32 = mybir.dt.float32
BF16 = mybir.dt.bfloat16
BF = ml_dtypes.bfloat16
P = 128
NK = IN_F // P  # 32 contraction chunks for GEMM1
RB = 512  # rows per block (one PSUM bank of fp32 y^T)
NB = ROWS // RB  # 2 blocks per core
KQ = 8  # k-chunks per load piece
NQ = NK // KQ  # 4 load pieces per block
HT = RB // P  # 4 row-tiles per block
ZC = 512  # GEMM2 moving chunk (one PSUM bank of fp32)
NJ = OUT_F // ZC  # 8 output chunks per row tile
NWARM = 10  # HAM warm-up matmuls (~4 us cold — one SHORT window)

_nc_cache = None


def build_nc() -> bass.Bass:
    nc = bacc.Bacc()
    # x^T, host-tiled: piece (b, q) occupies rows [(b*NQ+q)*P, +P), with
    # chunk kk of the piece at cols [kk*RB, (kk+1)*RB). Fully contiguous.
    xt_d = nc.declare_dram_parameter(
        "xt", [NB * NQ * P, KQ * RB], BF16, isOutput=False
    )
    # (2A)^T, partition-major chunk tiling: [128, 32*16]
    at_d = nc.declare_dram_parameter("at", [P, NK * R], BF16, isOutput=False)
    bb_d = nc.declare_dram_parameter("bb", [R + 1, OUT_F], BF16, isOutput=False)
    out_d = nc.declare_dram_parameter("out", [ROWS, OUT_F], BF16, isOutput=True)

    with TileContext(nc) as tc:
        with (
            tc.tile_pool(name="const", bufs=1) as const,
            tc.tile_pool(name="xin", bufs=3) as xin,
            tc.tile_pool(name="ytp", bufs=2) as ytp,
            tc.tile_pool(name="zrp", bufs=3) as zrp,
            tc.tile_pool(name="wpsum", bufs=1, space="PSUM") as wpsum,
            tc.tile_pool(name="ypsum", bufs=1, space="PSUM") as ypsum,
            tc.tile_pool(name="zpsum", bufs=6, space="PSUM") as zpsum,
        ):
            # HAM warm-up: keep the PE busy through one full activity window
            # while the first x piece loads, so real matmuls run at 2.4 GHz.
            wsrc = const.tile([P, ZC], BF16)
            nc.vector.memset(wsrc[:, :], 0.0)
            w_ps = wpsum.tile([P, ZC], FP32)
            for _ in range(NWARM):
                nc.tensor.matmul(
                    w_ps, lhsT=wsrc[:, :P], rhs=wsrc[:, :], start=True, stop=True
                )

            # Const loads on the ACT ring — keep the SP ring free for x.
            at_sb = const.tile([P, NK * R], BF16)
            nc.scalar.dma_start(out=at_sb[:, :], in_=at_d[:, :])
            bb = const.tile([R + 1, OUT_F], BF16)
            nc.scalar.dma_start(out=bb[:, :], in_=bb_d[:, :])

            def load_piece(nb, q):
                xt_p = xin.tile([P, KQ * RB], BF16, tag="x")
                nc.sync.dma_start(
                    out=xt_p[:, :],
                    in_=xt_d[(nb * NQ + q) * P : (nb * NQ + q + 1) * P, :],
                )
                return xt_p

            def g1_group(y_ps, piece, q):
                for kk in range(KQ):
                    k = q * KQ + kk
                    nc.tensor.matmul(
                        y_ps,
                        lhsT=at_sb[:, k * R : (k + 1) * R],
                        rhs=piece[:, kk * RB : (kk + 1) * RB],
                        start=(k == 0),
                        stop=(k == NK - 1),
                    )

            def make_yt(y_ps):
                # Ones-fill the whole tile (engines can't start at partition
                # 16), then overwrite rows 0:16 with y — row 16 keeps 1.0.
                yt_sb = ytp.tile([R + 1, RB], BF16, tag="yt")
                nc.vector.memset(yt_sb[:, :], 1.0)
                nc.scalar.copy(out=yt_sb[0:R, :], in_=y_ps)
                return yt_sb

            def g2_tile(yt_sb, nb, h):
                zrow = zrp.tile([P, OUT_F], BF16, tag="z")
                for j in range(NJ):
                    z_ps = zpsum.tile([P, ZC], FP32, tag="zz")
                    nc.tensor.matmul(
                        z_ps,
                        lhsT=yt_sb[:, h * P : (h + 1) * P],
                        rhs=bb[:, j * ZC : (j + 1) * ZC],
                        start=True,
                        stop=True,
                    )
                    dst = zrow[:, j * ZC : (j + 1) * ZC]
                    if j % 2 == 0:
                        nc.vector.tensor_copy(out=dst, in_=z_ps)
                    else:
                        nc.scalar.copy(out=dst, in_=z_ps)
                nc.scalar.dma_start(
                    out=out_d[(nb * HT + h) * P : (nb * HT + h + 1) * P, :],
                    in_=zrow[:, :],
                )

            # Block 0: load + GEMM1.
            p0 = [load_piece(0, q) for q in range(NQ)]
            y0 = ypsum.tile([R, RB], FP32, tag="y")
            for q in range(NQ):
                g1_group(y0, p0[q], q)
            yt0 = make_yt(y0)

            # Interleave block 0's GEMM2 tiles with block 1's loads/GEMM1 so
            # y(b1) completes right after its last piece lands and the final
            # stores start as early as possible.
            y1 = ypsum.tile([R, RB], FP32, tag="y")
            for q in range(NQ):
                p1q = load_piece(1, q)
                g2_tile(yt0, 0, q)
                g1_group(y1, p1q, q)
            yt1 = make_yt(y1)
            for h in range(HT):
                g2_tile(yt1, 1, h)

    nc.finalize()  # Bacc.finalize runs compile(): wait legalization + reg alloc
    return nc


def make_in_maps(x, lora_A, lora_B, bias):
    x2 = np.asarray(x, dtype=np.float32).reshape(B * S, IN_F)
    # (2A)^T [4096, 16] -> partition-major chunk tiling [128, 32*16]
    at = (np.asarray(lora_A, dtype=np.float32).T * SCALING).astype(BF)
    at = np.ascontiguousarray(
        at.reshape(NK, P, R).transpose(1, 0, 2).reshape(P, NK * R)
    )
    bbh = np.ascontiguousarray(
        np.concatenate(
            [
                np.asarray(lora_B, dtype=np.float32).T,
                np.asarray(bias, dtype=np.float32)[None, :],
            ],
            axis=0,
        ).astype(BF)
    )
    xb = x2.astype(BF)
    maps = []
    for s in np.split(xb, N_CORES, axis=0):
        # s [1024 rows, 4096] -> x^T [4096 = (q kk p), 1024 = (b r)]
        # -> piece-major [(b q p), (kk r)]
        xt = np.ascontiguousarray(
            s.T.reshape(NQ, KQ, P, NB, RB)
            .transpose(3, 0, 2, 1, 4)
            .reshape(NB * NQ * P, KQ * RB)
        )
        maps.append({"xt": xt, "at": at, "bb": bbh})
    return maps


def run(inputs: dict, trace: bool = False, **kw):
    global _nc_cache
    if _nc_cache is None:
        _nc_cache = build_nc()
    in_maps = make_in_maps(**inputs)
    res = run_bass_kernel_spmd(
        _nc_cache, in_maps, list(range(N_CORES)), trace=trace, **kw
    )
    out = (
        np.concatenate([res.results[i]["out"] for i in range(N_CORES)], axis=0)
        .astype(np.float32)
        .reshape(B, S, OUT_F)
    )
    return out, res


def kernel(**inputs) -> np.ndarray:
    out, _ = run(inputs)
    return out
